# revision 4
# baseline (speedup 1.0000x reference)
"""Distributed Taylor-series diffusion kernel for Trainium2 (8 NeuronCores).

Computes out[:, c] = expm(-t[c] * L) @ x[:, c] via a truncated Taylor series
    y = sum_{k=0}^{K} (-t)^k L^k x / k!
with K = 10 (remainder ~3e-11, far below fp32 noise of the order-25 reference).

Distribution: L is symmetric, so core j holds the column block
L[:, 768j:768(j+1)] resident in SBUF (18.9 MB) and computes the transposed
shard term_T[c, v] = (term.T @ Lblk)[c, v] of each Taylor term. Each step the
shard is block-transposed back to natural [v, c] layout on the vector engine
and all-gathered (48 KB) so every core has the full term as the next step's
stationary matmul operand. Matmuls run in float32r mode (fp32 storage,
~1.5e-4 matmul relative error, 4x the throughput of plain fp32).
"""

import os
import sys

sys.path.insert(0, "/opt/trn_rl_repo")

import numpy as np

import concourse.bass as bass
import concourse.mybir as mybir
import concourse.tile as tile
from concourse import bacc
from concourse.bass_utils import run_bass_kernel_spmd

F32 = mybir.dt.float32
F32R = mybir.dt.float32r

V = 6144
C = 16
N_CORES = 8
VS = V // N_CORES          # 768 columns of L per core
NUT = V // 128             # 48 u-tiles (contraction dim)
LOCT = VS // 128           # 6 u-tiles produced per core per step
K_STEPS = 10

TRACE = False
LAST_RESULT = None

_cached_nc = None


def _build():
    nc = bacc.Bacc("TRN2", target_bir_lowering=False, debug=False,
                   num_devices=N_CORES)

    L_in = nc.dram_tensor("L", [V, VS], F32R, kind="ExternalInput")
    x_in = nc.dram_tensor("x", [V, C], F32R, kind="ExternalInput")
    ts_in = nc.dram_tensor("ts", [K_STEPS, C], F32, kind="ExternalInput")
    out_d = nc.dram_tensor("out", [C, VS], F32, kind="ExternalOutput")

    Copy = mybir.ActivationFunctionType.Copy

    with tile.TileContext(nc) as tc:
        with (
            tc.tile_pool(name="Lp", bufs=1) as Lp,
            tc.tile_pool(name="natp", bufs=2) as natp,
            tc.tile_pool(name="stgp", bufs=2) as stgp,
            tc.tile_pool(name="Sp", bufs=2) as Sp,
            tc.tile_pool(name="accp", bufs=1) as accp,
            tc.tile_pool(name="tsp", bufs=1) as tsp,
            tc.tile_pool(name="psp", bufs=2, space="PSUM") as psp,
            tc.tile_pool(name="dram", bufs=2, space="DRAM") as dram,
        ):
            # ---- resident L: 48 tiles of [128, 768]
            Lt = []
            for u in range(NUT):
                lt = Lp.tile([128, VS], F32R, tag=f"L{u}")
                nc.sync.dma_start(lt[:], L_in[128 * u:128 * (u + 1), :])
                Lt.append(lt)

            # ---- per-step scale vectors: ts_sb[c, k] = -t_c / (k+1)
            ts_sb = tsp.tile([C, K_STEPS], F32)
            nc.sync.dma_start(ts_sb[:], ts_in[:].rearrange("k c -> c k"))

            # ---- accumulator (transposed shard), partitions 0:16 valid
            acc = accp.tile([32, VS], F32)
            nc.vector.memset(acc[:], 0.0)

            def new_nat():
                # natural-layout term, 8 rank blocks of [128, 6*32] (16 valid
                # cols per 32-col group)
                return [natp.tile([128, LOCT * 32], F32R, tag=f"nat{r}",
                                  name=f"nat{r}")
                        for r in range(N_CORES)]

            # ---- initial term = x (natural layout, from HBM)
            nat = new_nat()
            for r in range(N_CORES):
                nc.sync.dma_start(
                    nat[r][:].rearrange("p (i e) -> p i e", e=32)[:, :, 0:C],
                    x_in[VS * r:VS * (r + 1), :].rearrange(
                        "(i p) c -> p i c", p=128),
                )

            for k in range(1, K_STEPS + 1):
                # ---- term_T[c, v] = (term.T @ Lblk)[c, v], accumulated over
                # 48 u-tiles; psum bank A = v 0:512, bank B = v 512:768
                psA = psp.tile([16, 512], F32, tag="psA")
                psB = psp.tile([16, 256], F32, tag="psB")
                for ps, lo, n in ((psA, 0, 512), (psB, 512, 256)):
                    for u in range(NUT):
                        lhsT = nat[u // LOCT][:, (u % LOCT) * 32:
                                              (u % LOCT) * 32 + C]
                        nc.tensor.matmul(ps[:], lhsT, Lt[u][:, lo:lo + n],
                                         start=(u == 0), stop=(u == NUT - 1))

                # ---- scale by -t_c/k (per-partition scalar) on ScalarE
                S = Sp.tile([32, VS], F32R)
                nc.scalar.activation(S[0:C, 0:512], psA[:], Copy,
                                     scale=ts_sb[:, k - 1:k])
                nc.scalar.activation(S[0:C, 512:VS], psB[:], Copy,
                                     scale=ts_sb[:, k - 1:k])

                # ---- accumulate the Taylor term
                nc.vector.tensor_add(acc[0:C, :], acc[0:C, :], S[0:C, :])

                if k == K_STEPS:
                    break

                # ---- block-transpose shard to natural layout:
                # stg[32b + r2, 32(kk//4) + c] = S[c, 32kk + r2], kk = 0..23
                stg = stgp.tile([128, LOCT * 32], F32R, tag="stg")
                s_blocks = S[:].bitcast(F32).rearrange("p (kk e) -> p kk e",
                                                       e=32)
                for b in range(4):
                    nc.vector.transpose(
                        stg[32 * b:32 * (b + 1), :].bitcast(F32).rearrange(
                            "p (kk e) -> p kk e", e=32),
                        s_blocks[:, b::4, :],
                    )

                # ---- all-gather the natural-layout shard
                b_in = dram.tile([VS, C], F32R, tag="bin")
                b_out = dram.tile([V, C], F32R, tag="bout", addr_space="Shared")
                nc.sync.dma_start(
                    b_in[:].rearrange("(i p) c -> p i c", p=128),
                    stg[:].rearrange("p (i e) -> p i e", e=32)[:, :, 0:C],
                )
                nc.gpsimd.collective_compute(
                    "AllGather",
                    mybir.AluOpType.bypass,
                    replica_groups=[list(range(N_CORES))],
                    ins=[b_in.opt()],
                    outs=[b_out.opt()],
                )
                nat = new_nat()
                for r in range(N_CORES):
                    nc.sync.dma_start(
                        nat[r][:].rearrange("p (i e) -> p i e", e=32)[:, :, 0:C],
                        b_out[VS * r:VS * (r + 1), :].rearrange(
                            "(i p) c -> p i c", p=128),
                    )

            nc.sync.dma_start(out_d[:], acc[0:C, :])

    nc.compile()
    return nc


def _get_nc():
    global _cached_nc
    if _cached_nc is None:
        _cached_nc = _build()
    return _cached_nc


def kernel(x: np.ndarray, L: np.ndarray, t: np.ndarray) -> np.ndarray:
    global LAST_RESULT
    x = np.ascontiguousarray(np.asarray(x, dtype=np.float32))
    L = np.asarray(L, dtype=np.float32)
    t = np.asarray(t, dtype=np.float32)
    assert x.shape == (V, C) and L.shape == (V, V) and t.shape == (C,)

    tc_ = np.clip(t, 1e-8, None)
    ts = np.stack([-(tc_ / np.float32(k)) for k in range(1, K_STEPS + 1)])
    ts = np.ascontiguousarray(ts.astype(np.float32))

    in_maps = []
    for j in range(N_CORES):
        in_maps.append({
            "L": np.ascontiguousarray(L[:, VS * j:VS * (j + 1)]),
            "x": x,
            "ts": ts,
        })

    nc = _get_nc()
    res = run_bass_kernel_spmd(nc, in_maps, core_ids=list(range(N_CORES)),
                               trace=TRACE)
    LAST_RESULT = res

    y = np.empty((V, C), dtype=np.float32)
    for j in range(N_CORES):
        y[VS * j:VS * (j + 1), :] = res.results[j]["out"].T
    return x + y
